# revision 8
# baseline (speedup 1.0000x reference)
"""GCN + 2-step APPNP propagation on 8 Trainium2 NeuronCores.

Reference computation (N=16384, NFEAT=500, HIDDEN=32, NCLASS=3, alpha=0.25):
    h   = relu(input @ W1)
    l0  = h @ W2
    deg = adj.sum(axis=1);  d = (1 - alpha) / max(deg, 1e-12)
    l1  = d * (adj @ l0) + alpha * l0
    l2  = d * (adj @ l1) + alpha * l0
    out = log_softmax(l2, axis=1)

Distribution: 1D row partition of the graph. Core r owns rows
r*2048..(r+1)*2048.  The dominant cost is streaming adj (1 GiB fp32) twice.

Layout trick: TensorE contracts over the partition (SBUF row) axis, so
computing adj @ L needs adj's *column* index on partitions.  We therefore
ship each core T_r = adj[rows_r, :].T  (shape [N, 2048], row-major), cast to
bf16 on the host (halves HBM traffic; quantization error ~1e-4 relative).
A [128, 4] chunk of L is the stationary operand (LDWEIGHTS is ~free) and
T_r streams through as the moving operand at 1 column/cycle.

deg is fused into pass 1 as a fourth ones-column of L0.  Between the two
propagation passes the tiny per-core logits ([2048, 3]) are AllGathered
through a DRAM bounce.  Output is produced in a chunk-major layout
[128, 16, 3] per core and un-permuted on the host.
"""

import os

import numpy as np
import ml_dtypes

import concourse.bass as bass
import concourse.mybir as mybir
import concourse.bacc as bacc
import concourse.tile as tile
from concourse import bass_utils

N = 16384
NFEAT = 500
HIDDEN = 32
NCLASS = 3
ALPHA = 0.25
NCORES = 8
ROWS = N // NCORES        # 2048 rows owned per core
P = 128                   # SBUF partitions
CHUNKS = N // P           # 128 global row-chunks
LCH = ROWS // P           # 16 local row-chunks
NB = 4                    # row-chunks per adj DMA block ([128, 4, ROWS] tile)
ISL = 512                 # moving-operand free-dim per matmul
NISL = ROWS // ISL        # 4 output column slices

F32 = mybir.dt.float32
BF16 = mybir.dt.bfloat16
AF = mybir.ActivationFunctionType
ALU = mybir.AluOpType

_COMPILED = None
LAST_EXEC_TIME_NS = None
LAST_RESULTS = None


def _build():
    nc = bacc.Bacc("TRN2", target_bir_lowering=False, debug=False,
                   num_devices=NCORES)

    t_d = nc.dram_tensor("t", [N, ROWS], BF16, kind="ExternalInput").ap()
    xt_d = nc.dram_tensor("xt", [NFEAT, ROWS], F32, kind="ExternalInput").ap()
    w1_d = nc.dram_tensor("w1", [NFEAT, HIDDEN], F32, kind="ExternalInput").ap()
    w2_d = nc.dram_tensor("w2", [HIDDEN, NCLASS], F32, kind="ExternalInput").ap()
    eye_d = nc.dram_tensor("eye", [4, 4], F32, kind="ExternalInput").ap()
    out_d = nc.dram_tensor("out", [P, LCH * NCLASS], F32,
                           kind="ExternalOutput").ap()

    # adj^T shard viewed as DMA blocks: row = b*(NB*P) + s*P + p
    t_blk = t_d.rearrange("(b s p) f -> b p s f", p=P, s=NB)
    nblk = N // (NB * P)  # 32

    with tile.TileContext(nc) as tc:
        with (
            tc.tile_pool(name="const", bufs=1) as const,
            tc.tile_pool(name="persist", bufs=1) as persist,
            tc.tile_pool(name="dram", bufs=1, space="DRAM") as dram,
        ):
            eye_sb = const.tile([4, 4], F32)
            nc.sync.dma_start(eye_sb[:], eye_d[:])
            w2_sb = const.tile([HIDDEN, NCLASS], F32)
            nc.sync.dma_start(w2_sb[:], w2_d[:])

            # live across the whole kernel
            alpha_l0 = persist.tile([P, LCH * NCLASS], F32)   # 0.25*l0, local
            d_all = persist.tile([P, LCH], F32)               # 0.75/deg, local
            l0_rhs = persist.tile([P, CHUNKS, 4], BF16)       # [l0 | 1] chunks
            l1_rhs = persist.tile([P, CHUNKS, NCLASS], BF16)  # l1 chunks
            l0c = persist.tile([P, LCH, 4], BF16)             # local AG payload
            l1c = persist.tile([P, LCH, NCLASS], BF16)        # local AG payload
            out_sb = persist.tile([P, LCH * NCLASS], F32)

            # ---- stage 1: local l0 = relu(x @ W1) @ W2 (transposed forms) --
            ksz = [P, P, P, NFEAT - 3 * P]  # 500 = 128*3 + 116
            with (
                tc.tile_pool(name="s1sb", bufs=2) as s1sb,
                tc.tile_pool(name="s1w", bufs=1) as s1w,
                tc.tile_pool(name="hT", bufs=1) as hTp,
                tc.tile_pool(name="s1ps", bufs=2, space="PSUM") as s1ps,
            ):
                w1c = []
                for k in range(4):
                    w = s1w.tile([ksz[k], HIDDEN], F32, name=f"w1c{k}")
                    nc.sync.dma_start(w[:], w1_d[k * P:k * P + ksz[k], :])
                    w1c.append(w)
                xtc = []
                for k in range(4):
                    x = s1sb.tile([ksz[k], ROWS], F32, name=f"xtc{k}",
                                  tag=f"xtc{k}")
                    nc.sync.dma_start(x[:], xt_d[k * P:k * P + ksz[k], :])
                    xtc.append(x)

                hT = hTp.tile([HIDDEN, ROWS], F32)  # h^T, fp32 in SBUF
                for i in range(NISL):
                    hps = s1ps.tile([HIDDEN, ISL], F32, name=f"hps{i}",
                                    tag="hps")
                    for k in range(4):
                        nc.tensor.matmul(
                            hps[:], w1c[k][:],
                            xtc[k][:, i * ISL:(i + 1) * ISL],
                            start=(k == 0), stop=(k == 3))
                    nc.scalar.activation(hT[:, i * ISL:(i + 1) * ISL], hps[:],
                                         AF.Relu)

                # ones column for the fused degree computation
                nc.vector.memset(l0c[:, :, NCLASS], 1.0)

                for n in range(LCH):
                    lps = s1ps.tile([P, NCLASS], F32, name=f"lps{n}",
                                    tag="lps")
                    nc.tensor.matmul(lps[:], hT[:, n * P:(n + 1) * P],
                                     w2_sb[:], start=True, stop=True)
                    nc.vector.tensor_scalar_mul(
                        alpha_l0[:, n * NCLASS:(n + 1) * NCLASS], lps[:],
                        ALPHA)
                    nc.scalar.activation(l0c[:, n, 0:NCLASS], lps[:], AF.Copy)

            # ---- all-gather l0 (with ones col) into every core's rhs ------
            cc1_in = dram.tile([ROWS * 4], BF16)
            cc1_out = dram.tile([N * 4], BF16)
            nc.sync.dma_start(
                cc1_in[:].rearrange("(p f) -> p f", p=P),
                l0c[:].rearrange("p n f -> p (n f)"))
            nc.gpsimd.collective_compute(
                "AllGather", ALU.bypass,
                replica_groups=[list(range(NCORES))],
                ins=[cc1_in.opt()], outs=[cc1_out.opt()])
            nc.sync.dma_start(
                l0_rhs[:].rearrange("p c f -> p (c f)")
                .rearrange("p (k f) -> p k f", k=NCORES),
                cc1_out[:].rearrange("(k p f) -> p k f", k=NCORES, p=P))

            # ---- propagation pass 1: y1 = adj @ [l0 | 1] ------------------
            with (
                tc.tile_pool(name="tp1", bufs=3) as tp1,
                tc.tile_pool(name="y1ps", bufs=1, space="PSUM") as y1psp,
            ):
                y1ps = [y1psp.tile([4, ISL], F32, name=f"y1ps{i}",
                                   tag=f"y1ps{i}") for i in range(NISL)]
                for b in range(nblk):
                    tt = tp1.tile([P, NB, ROWS], BF16, name="tt1", tag="tt1")
                    nc.sync.dma_start(tt[:], t_blk[b])
                    for s in range(NB):
                        jc = b * NB + s
                        lhsT = l0_rhs[:, jc, :]
                        for i in range(NISL):
                            nc.tensor.matmul(
                                y1ps[i][:], lhsT,
                                tt[:, s, i * ISL:(i + 1) * ISL],
                                start=(jc == 0), stop=(jc == CHUNKS - 1))

                y1T = persist.tile([4, ROWS], F32)
                for i in range(NISL):
                    nc.scalar.activation(y1T[:, i * ISL:(i + 1) * ISL],
                                         y1ps[i][:], AF.Copy)

            # ---- iteration update: l1 = d*y1 + alpha*l0 -------------------
            with (
                tc.tile_pool(name="upd", bufs=2) as upd,
                tc.tile_pool(name="updps", bufs=2, space="PSUM") as updps,
            ):
                for n in range(LCH):
                    ytp = updps.tile([P, 4], F32, name=f"ytp{n}", tag="ytp")
                    nc.tensor.transpose(ytp[:], y1T[:, n * P:(n + 1) * P],
                                        eye_sb[:])
                    dmx = upd.tile([P, 1], F32, name=f"dmx{n}", tag="dmx")
                    nc.vector.tensor_scalar_max(dmx[:], ytp[:, 3:4], 1e-12)
                    rec = upd.tile([P, 1], F32, name=f"rec{n}", tag="rec")
                    nc.vector.reciprocal(rec[:], dmx[:])
                    nc.vector.tensor_scalar_mul(d_all[:, n:n + 1], rec[:],
                                                1.0 - ALPHA)
                    ty = upd.tile([P, NCLASS], F32, name=f"ty{n}", tag="ty")
                    nc.vector.tensor_scalar_mul(ty[:], ytp[:, 0:NCLASS],
                                                d_all[:, n:n + 1])
                    nc.vector.tensor_add(
                        l1c[:, n, :], ty[:],
                        alpha_l0[:, NCLASS * n:NCLASS * (n + 1)])

            # ---- all-gather l1 --------------------------------------------
            cc2_in = dram.tile([ROWS * NCLASS], BF16)
            cc2_out = dram.tile([N * NCLASS], BF16)
            nc.sync.dma_start(
                cc2_in[:].rearrange("(p f) -> p f", p=P),
                l1c[:].rearrange("p n f -> p (n f)"))
            nc.gpsimd.collective_compute(
                "AllGather", ALU.bypass,
                replica_groups=[list(range(NCORES))],
                ins=[cc2_in.opt()], outs=[cc2_out.opt()])
            nc.sync.dma_start(
                l1_rhs[:].rearrange("p c f -> p (c f)")
                .rearrange("p (k f) -> p k f", k=NCORES),
                cc2_out[:].rearrange("(k p f) -> p k f", k=NCORES, p=P))

            # ---- propagation pass 2: y2 = adj @ l1 ------------------------
            with (
                tc.tile_pool(name="tp2", bufs=3) as tp2,
                tc.tile_pool(name="y2ps", bufs=1, space="PSUM") as y2psp,
            ):
                y2ps = [y2psp.tile([NCLASS, ISL], F32, name=f"y2ps{i}",
                                   tag=f"y2ps{i}") for i in range(NISL)]
                for b in range(nblk):
                    tt = tp2.tile([P, NB, ROWS], BF16, name="tt2", tag="tt2")
                    nc.sync.dma_start(tt[:], t_blk[b])
                    for s in range(NB):
                        jc = b * NB + s
                        lhsT = l1_rhs[:, jc, :]
                        for i in range(NISL):
                            nc.tensor.matmul(
                                y2ps[i][:], lhsT,
                                tt[:, s, i * ISL:(i + 1) * ISL],
                                start=(jc == 0), stop=(jc == CHUNKS - 1))

                y2T = persist.tile([NCLASS, ROWS], F32)
                for i in range(NISL):
                    nc.scalar.activation(y2T[:, i * ISL:(i + 1) * ISL],
                                         y2ps[i][:], AF.Copy)

            # ---- final update + log_softmax -------------------------------
            with (
                tc.tile_pool(name="fin", bufs=2) as fin,
                tc.tile_pool(name="finps", bufs=2, space="PSUM") as finps,
            ):
                for n in range(LCH):
                    ytp = finps.tile([P, NCLASS], F32, name=f"fyt{n}",
                                     tag="fyt")
                    nc.tensor.transpose(ytp[:], y2T[:, n * P:(n + 1) * P],
                                        eye_sb[0:NCLASS, 0:NCLASS])
                    lg = fin.tile([P, NCLASS], F32, name=f"lg{n}", tag="lg")
                    nc.vector.tensor_scalar_mul(lg[:], ytp[:],
                                                d_all[:, n:n + 1])
                    nc.vector.tensor_add(
                        lg[:], lg[:],
                        alpha_l0[:, NCLASS * n:NCLASS * (n + 1)])
                    negm = fin.tile([P, 1], F32, name=f"negm{n}", tag="negm")
                    nc.vector.tensor_reduce(negm[:], lg[:],
                                            axis=mybir.AxisListType.X,
                                            op=ALU.max, negate=True)
                    ex = fin.tile([P, NCLASS], F32, name=f"ex{n}", tag="ex")
                    nc.scalar.activation(ex[:], lg[:], AF.Exp,
                                         bias=negm[:, 0:1])
                    sm = fin.tile([P, 1], F32, name=f"sm{n}", tag="sm")
                    nc.vector.tensor_reduce(sm[:], ex[:],
                                            axis=mybir.AxisListType.X,
                                            op=ALU.add)
                    rs = fin.tile([P, 1], F32, name=f"rs{n}", tag="rs")
                    nc.vector.reciprocal(rs[:], sm[:])
                    nls = fin.tile([P, 1], F32, name=f"nls{n}", tag="nls")
                    nc.scalar.activation(nls[:], rs[:], AF.Ln)
                    nc.vector.tensor_scalar(
                        out_sb[:, NCLASS * n:NCLASS * (n + 1)], lg[:],
                        negm[:, 0:1], nls[:, 0:1], ALU.add, ALU.add)

            nc.sync.dma_start(out_d[:], out_sb[:])

    nc.compile()
    return nc


def kernel(input, adj, W1, W2):
    """Full inputs in, full [N, NCLASS] float32 log-softmax out."""
    global _COMPILED, LAST_EXEC_TIME_NS, LAST_RESULTS
    if _COMPILED is None:
        _COMPILED = _build()
    nc = _COMPILED

    input = np.asarray(input, dtype=np.float32)
    adj = np.asarray(adj, dtype=np.float32)
    W1 = np.asarray(W1, dtype=np.float32)
    W2 = np.asarray(W2, dtype=np.float32)

    adj_bf = adj.astype(ml_dtypes.bfloat16)
    xt = np.ascontiguousarray(input.T)
    eye = np.eye(4, dtype=np.float32)

    in_maps = []
    for r in range(NCORES):
        t_r = np.ascontiguousarray(adj_bf[r * ROWS:(r + 1) * ROWS, :].T)
        in_maps.append({
            "t": t_r,
            "xt": np.ascontiguousarray(xt[:, r * ROWS:(r + 1) * ROWS]),
            "w1": W1,
            "w2": W2,
            "eye": eye,
        })

    res = bass_utils.run_bass_kernel_spmd(
        nc, in_maps, core_ids=list(range(NCORES)),
        trace=bool(os.environ.get("GNN_TRACE")))
    LAST_EXEC_TIME_NS = res.exec_time_ns
    LAST_RESULTS = res

    out = np.empty((N, NCLASS), dtype=np.float32)
    for r in range(NCORES):
        blk = res.results[r]["out"].reshape(P, LCH, NCLASS)
        out[r * ROWS:(r + 1) * ROWS] = (
            blk.transpose(1, 0, 2).reshape(ROWS, NCLASS))
    return out


# revision 9
# speedup vs baseline: 1.5586x; 1.5586x over previous
"""GCN + 2-step APPNP propagation on 8 Trainium2 NeuronCores.

Reference computation (N=16384, NFEAT=500, HIDDEN=32, NCLASS=3, alpha=0.25):
    h   = relu(input @ W1)
    l0  = h @ W2
    deg = adj.sum(axis=1);  d = (1 - alpha) / max(deg, 1e-12)
    l1  = d * (adj @ l0) + alpha * l0
    l2  = d * (adj @ l1) + alpha * l0
    out = log_softmax(l2, axis=1)

Distribution: 1D row partition of the graph; core r owns rows
r*2048..(r+1)*2048.  The dominant cost is streaming adj twice.

Layout: TensorE contracts over the partition axis, so adj @ L needs adj's
column index on partitions; each core gets T_r = adj[rows_r, :].T
([N, 2048] row-major), quantized to fp8-e4m3 on the host (4x less HBM
traffic than fp32; measured output error ~1e-4 relative because the
propagated term is small next to the fp32 alpha*l0 term and quantization
noise averages over 16k-term dot products).  A [128, c] chunk of L is the
stationary operand (LDWEIGHTS ~free); T_r streams as the moving operand.

deg rides along pass 1 as a ones-column of L0.  Between passes the tiny
per-core logits are AllGathered through a DRAM bounce.  Small/latency-
critical DMAs go on the scalar-engine HWDGE queue so they never sit
behind the 2 MiB stream DMAs on the sync queue.  Output leaves in a
chunk-major [128, 16, 3] layout and is un-permuted on the host.
"""

import os

import numpy as np
import ml_dtypes

import concourse.bass as bass
import concourse.mybir as mybir
import concourse.bacc as bacc
import concourse.tile as tile
from concourse import bass_utils

N = 16384
NFEAT = 500
HIDDEN = 32
NCLASS = 3
ALPHA = 0.25
NCORES = 8
ROWS = N // NCORES        # 2048 rows owned per core
P = 128                   # SBUF partitions
CHUNKS = N // P           # 128 global row-chunks
LCH = ROWS // P           # 16 local row-chunks
NB = 8                    # row-chunks per adj DMA block
ISL = 512                 # moving-operand free-dim per matmul
NISL = ROWS // ISL        # 4 output column slices
TT_BUFS = 7               # adj stream prefetch depth (x2 MiB)

F32 = mybir.dt.float32
ADT = mybir.dt.float8e4
ADT_NP = ml_dtypes.float8_e4m3
AF = mybir.ActivationFunctionType
ALU = mybir.AluOpType
AX = mybir.AxisListType

_COMPILED = None
LAST_EXEC_TIME_NS = None
LAST_RESULTS = None


def _build():
    nc = bacc.Bacc("TRN2", target_bir_lowering=False, debug=False,
                   num_devices=NCORES)

    t_d = nc.dram_tensor("t", [N, ROWS], ADT, kind="ExternalInput").ap()
    xt_d = nc.dram_tensor("xt", [NFEAT, ROWS], F32, kind="ExternalInput").ap()
    w1_d = nc.dram_tensor("w1", [NFEAT, HIDDEN], F32, kind="ExternalInput").ap()
    w2_d = nc.dram_tensor("w2", [HIDDEN, NCLASS], F32, kind="ExternalInput").ap()
    eye_d = nc.dram_tensor("eye", [4, 4], F32, kind="ExternalInput").ap()
    out_d = nc.dram_tensor("out", [P, LCH * NCLASS], F32,
                           kind="ExternalOutput").ap()

    # adj^T shard viewed as DMA blocks: row = b*(NB*P) + s*P + p
    t_blk = t_d.rearrange("(b s p) f -> b p s f", p=P, s=NB)
    nblk = N // (NB * P)  # 16

    rg = [list(range(NCORES))]

    with tile.TileContext(nc) as tc:
        with (
            tc.tile_pool(name="const", bufs=1) as const,
            tc.tile_pool(name="persist", bufs=1) as persist,
            tc.tile_pool(name="ttp", bufs=TT_BUFS) as ttp,
            tc.tile_pool(name="dram", bufs=1, space="DRAM") as dram,
        ):
            eye_sb = const.tile([4, 4], F32)
            nc.scalar.dma_start(eye_sb[:], eye_d[:])
            w2_sb = const.tile([HIDDEN, NCLASS], F32)
            nc.scalar.dma_start(w2_sb[:], w2_d[:])

            # live across the whole kernel
            alpha_l0 = persist.tile([P, LCH, NCLASS], F32)    # 0.25*l0, local
            d_all = persist.tile([P, LCH], F32)               # 0.75/deg, local
            l0_rhs = persist.tile([P, CHUNKS, 4], ADT)        # [l0 | 1] chunks
            l1_rhs = persist.tile([P, CHUNKS, NCLASS], ADT)   # l1 chunks
            l0c = persist.tile([P, LCH, 4], ADT)              # local AG payload
            l1c = persist.tile([P, LCH, NCLASS], ADT)         # local AG payload
            out_sb = persist.tile([P, LCH, NCLASS], F32)

            # ---- stage 1: local l0 = relu(x @ W1) @ W2 (transposed forms) --
            ksz = [P, P, P, NFEAT - 3 * P]  # 500 = 128*3 + 116
            with (
                tc.tile_pool(name="s1sb", bufs=1) as s1sb,
                tc.tile_pool(name="s1ps", bufs=2, space="PSUM") as s1ps,
                tc.tile_pool(name="l0psp", bufs=1, space="PSUM") as l0psp,
            ):
                w1c, xtc = [], []
                for k in range(4):
                    w = s1sb.tile([ksz[k], HIDDEN], F32, name=f"w1c{k}")
                    nc.scalar.dma_start(w[:], w1_d[k * P:k * P + ksz[k], :])
                    w1c.append(w)
                for k in range(4):
                    x = s1sb.tile([ksz[k], ROWS], F32, name=f"xtc{k}")
                    nc.scalar.dma_start(x[:], xt_d[k * P:k * P + ksz[k], :])
                    xtc.append(x)

                hT = s1sb.tile([HIDDEN, ROWS], F32)  # h^T in SBUF
                for i in range(NISL):
                    hps = s1ps.tile([HIDDEN, ISL], F32, name=f"hps{i}",
                                    tag="hps")
                    for k in range(4):
                        nc.tensor.matmul(
                            hps[:], w1c[k][:],
                            xtc[k][:, i * ISL:(i + 1) * ISL],
                            start=(k == 0), stop=(k == 3))
                    nc.scalar.activation(hT[:, i * ISL:(i + 1) * ISL], hps[:],
                                         AF.Relu)

                l0ps = l0psp.tile([P, LCH, NCLASS], F32)
                for n in range(LCH):
                    nc.tensor.matmul(l0ps[:, n, :], hT[:, n * P:(n + 1) * P],
                                     w2_sb[:], start=True, stop=True)
                nc.vector.tensor_scalar_mul(alpha_l0[:], l0ps[:], ALPHA)
                nc.scalar.activation(l0c[:, :, 0:NCLASS], l0ps[:], AF.Copy)
                nc.vector.memset(l0c[:, :, NCLASS], 1.0)

            # ---- all-gather l0 (with ones col) into every core's rhs ------
            cc1_in = dram.tile([ROWS * 4], ADT)
            cc1_out = dram.tile([N * 4], ADT)
            nc.scalar.dma_start(
                cc1_in[:].rearrange("(p f) -> p f", p=P),
                l0c[:].rearrange("p n f -> p (n f)"))
            nc.gpsimd.collective_compute(
                "AllGather", ALU.bypass, replica_groups=rg,
                ins=[cc1_in.opt()], outs=[cc1_out.opt()])
            nc.scalar.dma_start(
                l0_rhs[:].rearrange("p c f -> p (c f)")
                .rearrange("p (k f) -> p k f", k=NCORES),
                cc1_out[:].rearrange("(k p f) -> p k f", k=NCORES, p=P))

            # ---- propagation pass 1: y1 = adj @ [l0 | 1] ------------------
            with tc.tile_pool(name="y1ps", bufs=1, space="PSUM") as y1psp:
                y1ps = [y1psp.tile([4, ISL], F32, name=f"y1ps{i}",
                                   tag=f"y1ps{i}") for i in range(NISL)]
                for b in range(nblk):
                    tt = ttp.tile([P, NB, ROWS], ADT, name="tt", tag="tt")
                    nc.sync.dma_start(tt[:], t_blk[b])
                    for s in range(NB):
                        jc = b * NB + s
                        for i in range(NISL):
                            nc.tensor.matmul(
                                y1ps[i][:], l0_rhs[:, jc, :],
                                tt[:, s, i * ISL:(i + 1) * ISL],
                                start=(jc == 0), stop=(jc == CHUNKS - 1))

                y1T = persist.tile([4, ROWS], F32)
                for i in range(NISL):
                    nc.scalar.activation(y1T[:, i * ISL:(i + 1) * ISL],
                                         y1ps[i][:], AF.Copy)

            # ---- iteration update: l1 = d*y1 + alpha*l0 -------------------
            with (
                tc.tile_pool(name="upd", bufs=1) as upd,
                tc.tile_pool(name="updps", bufs=1, space="PSUM") as updps,
            ):
                ytp = updps.tile([P, LCH, 4], F32)
                for n in range(LCH):
                    nc.tensor.transpose(ytp[:, n, :],
                                        y1T[:, n * P:(n + 1) * P], eye_sb[:])
                dmx = upd.tile([P, LCH], F32)
                nc.vector.tensor_scalar_max(dmx[:], ytp[:, :, 3], 1e-12)
                rec = upd.tile([P, LCH], F32)
                nc.vector.reciprocal(rec[:], dmx[:])
                nc.vector.tensor_scalar_mul(d_all[:], rec[:], 1.0 - ALPHA)
                ty = upd.tile([P, LCH, NCLASS], F32)
                nc.vector.tensor_mul(ty[:], ytp[:, :, 0:NCLASS],
                                     d_all[:].broadcast_to([P, LCH, NCLASS]))
                nc.vector.tensor_add(l1c[:], ty[:], alpha_l0[:])

            # ---- all-gather l1 --------------------------------------------
            cc2_in = dram.tile([ROWS * NCLASS], ADT)
            cc2_out = dram.tile([N * NCLASS], ADT)
            nc.scalar.dma_start(
                cc2_in[:].rearrange("(p f) -> p f", p=P),
                l1c[:].rearrange("p n f -> p (n f)"))
            nc.gpsimd.collective_compute(
                "AllGather", ALU.bypass, replica_groups=rg,
                ins=[cc2_in.opt()], outs=[cc2_out.opt()])
            nc.scalar.dma_start(
                l1_rhs[:].rearrange("p c f -> p (c f)")
                .rearrange("p (k f) -> p k f", k=NCORES),
                cc2_out[:].rearrange("(k p f) -> p k f", k=NCORES, p=P))

            # ---- propagation pass 2: y2 = adj @ l1 ------------------------
            with tc.tile_pool(name="y2ps", bufs=1, space="PSUM") as y2psp:
                y2ps = [y2psp.tile([NCLASS, ISL], F32, name=f"y2ps{i}",
                                   tag=f"y2ps{i}") for i in range(NISL)]
                for b in range(nblk):
                    tt = ttp.tile([P, NB, ROWS], ADT, name="tt", tag="tt")
                    nc.sync.dma_start(tt[:], t_blk[b])
                    for s in range(NB):
                        jc = b * NB + s
                        for i in range(NISL):
                            nc.tensor.matmul(
                                y2ps[i][:], l1_rhs[:, jc, :],
                                tt[:, s, i * ISL:(i + 1) * ISL],
                                start=(jc == 0), stop=(jc == CHUNKS - 1))

                y2T = persist.tile([NCLASS, ROWS], F32)
                for i in range(NISL):
                    nc.scalar.activation(y2T[:, i * ISL:(i + 1) * ISL],
                                         y2ps[i][:], AF.Copy)

            # ---- final update + log_softmax -------------------------------
            with (
                tc.tile_pool(name="fin", bufs=1) as fin,
                tc.tile_pool(name="finps", bufs=1, space="PSUM") as finps,
            ):
                y2tp = finps.tile([P, LCH, NCLASS], F32)
                for n in range(LCH):
                    nc.tensor.transpose(y2tp[:, n, :],
                                        y2T[:, n * P:(n + 1) * P],
                                        eye_sb[0:NCLASS, 0:NCLASS])
                lg = fin.tile([P, LCH, NCLASS], F32)
                nc.vector.tensor_mul(lg[:], y2tp[:],
                                     d_all[:].broadcast_to([P, LCH, NCLASS]))
                nc.vector.tensor_add(lg[:], lg[:], alpha_l0[:])
                negm = fin.tile([P, LCH], F32)
                nc.vector.tensor_reduce(negm[:], lg[:], axis=AX.X, op=ALU.max,
                                        negate=True)
                lgm = fin.tile([P, LCH, NCLASS], F32)
                nc.vector.tensor_add(lgm[:], lg[:],
                                     negm[:].broadcast_to([P, LCH, NCLASS]))
                ex = fin.tile([P, LCH, NCLASS], F32)
                nc.scalar.activation(ex[:], lgm[:], AF.Exp)
                sm = fin.tile([P, LCH], F32)
                nc.vector.tensor_reduce(sm[:], ex[:], axis=AX.X, op=ALU.add)
                rs = fin.tile([P, LCH], F32)
                nc.vector.reciprocal(rs[:], sm[:])
                nls = fin.tile([P, LCH], F32)
                nc.scalar.activation(nls[:], rs[:], AF.Ln)
                nc.vector.tensor_add(out_sb[:], lgm[:],
                                     nls[:].broadcast_to([P, LCH, NCLASS]))

            nc.scalar.dma_start(out_d[:],
                                out_sb[:].rearrange("p n f -> p (n f)"))

    nc.compile()
    return nc


def kernel(input, adj, W1, W2):
    """Full inputs in, full [N, NCLASS] float32 log-softmax out."""
    global _COMPILED, LAST_EXEC_TIME_NS, LAST_RESULTS
    if _COMPILED is None:
        _COMPILED = _build()
    nc = _COMPILED

    input = np.asarray(input, dtype=np.float32)
    adj = np.asarray(adj, dtype=np.float32)
    W1 = np.asarray(W1, dtype=np.float32)
    W2 = np.asarray(W2, dtype=np.float32)

    adj_q = adj.astype(ADT_NP)
    xt = np.ascontiguousarray(input.T)
    eye = np.eye(4, dtype=np.float32)

    in_maps = []
    for r in range(NCORES):
        t_r = np.ascontiguousarray(adj_q[r * ROWS:(r + 1) * ROWS, :].T)
        in_maps.append({
            "t": t_r,
            "xt": np.ascontiguousarray(xt[:, r * ROWS:(r + 1) * ROWS]),
            "w1": W1,
            "w2": W2,
            "eye": eye,
        })

    res = bass_utils.run_bass_kernel_spmd(
        nc, in_maps, core_ids=list(range(NCORES)),
        trace=bool(os.environ.get("GNN_TRACE")))
    LAST_EXEC_TIME_NS = res.exec_time_ns
    LAST_RESULTS = res

    out = np.empty((N, NCLASS), dtype=np.float32)
    for r in range(NCORES):
        blk = res.results[r]["out"].reshape(P, LCH, NCLASS)
        out[r * ROWS:(r + 1) * ROWS] = (
            blk.transpose(1, 0, 2).reshape(ROWS, NCLASS))
    return out


# revision 10
# speedup vs baseline: 1.8521x; 1.1883x over previous
"""GCN + 2-step APPNP propagation on 8 Trainium2 NeuronCores.

Reference computation (N=16384, NFEAT=500, HIDDEN=32, NCLASS=3, alpha=0.25):
    h   = relu(input @ W1)
    l0  = h @ W2
    deg = adj.sum(axis=1);  d = (1 - alpha) / max(deg, 1e-12)
    l1  = d * (adj @ l0) + alpha * l0
    l2  = d * (adj @ l1) + alpha * l0
    out = log_softmax(l2, axis=1)

Distribution: 1D row partition of the graph; core r owns rows
r*2048..(r+1)*2048.  The dominant cost is streaming adj twice.

Layout: TensorE contracts over the partition axis, so adj @ L needs adj's
column index on partitions; each core gets T_r = adj[rows_r, :].T
([N, 2048] row-major), quantized to fp8-e4m3 on the host (4x less HBM
traffic than fp32; measured output error ~1e-4 relative because the
propagated term is small next to the fp32 alpha*l0 term and quantization
noise averages over 16k-term dot products).  A [128, c] chunk of L is the
stationary operand (LDWEIGHTS ~free); T_r streams as the moving operand.

deg rides along pass 1 as a ones-column of L0.  Between passes the tiny
per-core logits are AllGathered through a DRAM bounce.  Small/latency-
critical DMAs go on the scalar-engine HWDGE queue so they never sit
behind the 2 MiB stream DMAs on the sync queue.  Output leaves in a
chunk-major [128, 16, 3] layout and is un-permuted on the host.
"""

import os

import numpy as np
import ml_dtypes

import concourse.bass as bass
import concourse.mybir as mybir
import concourse.bacc as bacc
import concourse.tile as tile
from concourse import bass_utils

N = 16384
NFEAT = 500
HIDDEN = 32
NCLASS = 3
ALPHA = 0.25
NCORES = 8
ROWS = N // NCORES        # 2048 rows owned per core
P = 128                   # SBUF partitions
CHUNKS = N // P           # 128 global row-chunks
LCH = ROWS // P           # 16 local row-chunks
NB = 8                    # row-chunks per adj DMA block
ISL = 512                 # moving-operand free-dim per matmul
NISL = ROWS // ISL        # 4 output column slices
TT_BUFS = 8               # adj stream prefetch depth (x2 MiB)
LPAD = 16                 # L-chunk stride (DoubleRow needs step%16==0)

F32 = mybir.dt.float32
ADT = mybir.dt.float8e4
ADT_NP = ml_dtypes.float8_e4m3
AF = mybir.ActivationFunctionType
ALU = mybir.AluOpType
AX = mybir.AxisListType

_COMPILED = None
LAST_EXEC_TIME_NS = None
LAST_RESULTS = None


def _build():
    nc = bacc.Bacc("TRN2", target_bir_lowering=False, debug=False,
                   num_devices=NCORES)

    t_d = nc.dram_tensor("t", [N, ROWS], ADT, kind="ExternalInput").ap()
    xt_d = nc.dram_tensor("xt", [NFEAT, ROWS], F32, kind="ExternalInput").ap()
    w1_d = nc.dram_tensor("w1", [NFEAT, HIDDEN], F32, kind="ExternalInput").ap()
    w2_d = nc.dram_tensor("w2", [HIDDEN, NCLASS], F32, kind="ExternalInput").ap()
    eye_d = nc.dram_tensor("eye", [4, 4], F32, kind="ExternalInput").ap()
    out_d = nc.dram_tensor("out", [P, LCH * NCLASS], F32,
                           kind="ExternalOutput").ap()

    # adj^T shard viewed as DMA blocks: row = b*(NB*P) + s*P + p
    t_blk = t_d.rearrange("(b s p) f -> b p s f", p=P, s=NB)
    nblk = N // (NB * P)  # 16

    rg = [list(range(NCORES))]

    with tile.TileContext(nc) as tc:
        with (
            tc.tile_pool(name="const", bufs=1) as const,
            tc.tile_pool(name="persist", bufs=1) as persist,
            tc.tile_pool(name="ttp", bufs=TT_BUFS) as ttp,
            tc.tile_pool(name="dram", bufs=1, space="DRAM") as dram,
        ):
            eye_sb = const.tile([4, 4], F32)
            nc.scalar.dma_start(eye_sb[:], eye_d[:])
            w2_sb = const.tile([HIDDEN, NCLASS], F32)
            nc.scalar.dma_start(w2_sb[:], w2_d[:])

            # live across the whole kernel
            alpha_l0 = persist.tile([P, LCH, NCLASS], F32)    # 0.25*l0, local
            d_all = persist.tile([P, LCH], F32)               # 0.75/deg, local
            l0_rhs = persist.tile([P, CHUNKS, LPAD], ADT)     # [l0 | 1] chunks
            l1_rhs = persist.tile([P, CHUNKS, LPAD], ADT)     # l1 chunks
            l0c = persist.tile([P, LCH, LPAD], ADT)           # local AG payload
            l1c = persist.tile([P, LCH, LPAD], ADT)           # local AG payload
            out_sb = persist.tile([P, LCH, NCLASS], F32)

            # ---- stage 1: local l0 = relu(x @ W1) @ W2 (transposed forms) --
            ksz = [P, P, P, NFEAT - 3 * P]  # 500 = 128*3 + 116
            with (
                tc.tile_pool(name="s1sb", bufs=1) as s1sb,
                tc.tile_pool(name="s1ps", bufs=2, space="PSUM") as s1ps,
                tc.tile_pool(name="l0psp", bufs=1, space="PSUM") as l0psp,
            ):
                w1c, xtc = [], []
                for k in range(4):
                    w = s1sb.tile([ksz[k], HIDDEN], F32, name=f"w1c{k}")
                    nc.scalar.dma_start(w[:], w1_d[k * P:k * P + ksz[k], :])
                    w1c.append(w)
                for k in range(4):
                    x = s1sb.tile([ksz[k], ROWS], F32, name=f"xtc{k}")
                    nc.scalar.dma_start(x[:], xt_d[k * P:k * P + ksz[k], :])
                    xtc.append(x)

                hT = s1sb.tile([HIDDEN, ROWS], F32)  # h^T in SBUF
                for i in range(NISL):
                    hps = s1ps.tile([HIDDEN, ISL], F32, name=f"hps{i}",
                                    tag="hps")
                    for k in range(4):
                        nc.tensor.matmul(
                            hps[:], w1c[k][:],
                            xtc[k][:, i * ISL:(i + 1) * ISL],
                            start=(k == 0), stop=(k == 3))
                    nc.scalar.activation(hT[:, i * ISL:(i + 1) * ISL], hps[:],
                                         AF.Relu)

                l0ps = l0psp.tile([P, LCH, NCLASS], F32)
                for n in range(LCH):
                    nc.tensor.matmul(l0ps[:, n, :], hT[:, n * P:(n + 1) * P],
                                     w2_sb[:], start=True, stop=True)
                nc.vector.tensor_scalar_mul(alpha_l0[:], l0ps[:], ALPHA)
                nc.scalar.activation(l0c[:, :, 0:NCLASS], l0ps[:], AF.Copy)
                nc.vector.memset(l0c[:, :, NCLASS], 1.0)
                nc.vector.memset(l0c[:, :, NCLASS + 1:LPAD], 0.0)

            # ---- all-gather l0 (with ones col) into every core's rhs ------
            cc1_in = dram.tile([ROWS * LPAD], ADT)
            cc1_out = dram.tile([N * LPAD], ADT)
            nc.scalar.dma_start(
                cc1_in[:].rearrange("(p f) -> p f", p=P),
                l0c[:].rearrange("p n f -> p (n f)"))
            nc.gpsimd.collective_compute(
                "AllGather", ALU.bypass, replica_groups=rg,
                ins=[cc1_in.opt()], outs=[cc1_out.opt()])
            nc.scalar.dma_start(
                l0_rhs[:].rearrange("p c f -> p (c f)")
                .rearrange("p (k f) -> p k f", k=NCORES),
                cc1_out[:].rearrange("(k p f) -> p k f", k=NCORES, p=P))

            # ---- propagation pass 1: y1 = adj @ [l0 | 1] ------------------
            with tc.tile_pool(name="y1ps", bufs=1, space="PSUM") as y1psp:
                y1ps = [y1psp.tile([4, ISL], F32, name=f"y1ps{i}",
                                   tag=f"y1ps{i}") for i in range(NISL)]
                for b in range(nblk):
                    tt = ttp.tile([P, NB, ROWS], ADT, name="tt", tag="tt")
                    nc.sync.dma_start(tt[:], t_blk[b])
                    for s2 in range(NB // 2):
                        jc = b * NB + 2 * s2
                        for i in range(NISL):
                            nc.tensor.matmul(
                                y1ps[i][:], l0_rhs[:, jc:jc + 2, 0:4],
                                tt[:, 2 * s2:2 * s2 + 2,
                                   i * ISL:(i + 1) * ISL],
                                start=(jc == 0), stop=(jc == CHUNKS - 2),
                                perf_mode=mybir.MatmulPerfMode.DoubleRow)

                y1T = persist.tile([4, ROWS], F32)
                for i in range(NISL):
                    nc.scalar.activation(y1T[:, i * ISL:(i + 1) * ISL],
                                         y1ps[i][:], AF.Copy)

            # ---- iteration update: l1 = d*y1 + alpha*l0 -------------------
            with (
                tc.tile_pool(name="upd", bufs=1) as upd,
                tc.tile_pool(name="updps", bufs=1, space="PSUM") as updps,
            ):
                ytp = updps.tile([P, LCH, 4], F32)
                for n in range(LCH):
                    nc.tensor.transpose(ytp[:, n, :],
                                        y1T[:, n * P:(n + 1) * P], eye_sb[:])
                dmx = upd.tile([P, LCH], F32)
                nc.vector.tensor_scalar_max(dmx[:], ytp[:, :, 3], 1e-12)
                rec = upd.tile([P, LCH], F32)
                nc.vector.reciprocal(rec[:], dmx[:])
                nc.vector.tensor_scalar_mul(d_all[:], rec[:], 1.0 - ALPHA)
                ty = upd.tile([P, LCH, NCLASS], F32)
                nc.vector.tensor_mul(ty[:], ytp[:, :, 0:NCLASS],
                                     d_all[:].broadcast_to([P, LCH, NCLASS]))
                nc.vector.tensor_add(l1c[:, :, 0:NCLASS], ty[:],
                                     alpha_l0[:])
                nc.vector.memset(l1c[:, :, NCLASS:LPAD], 0.0)

            # ---- all-gather l1 --------------------------------------------
            cc2_in = dram.tile([ROWS * LPAD], ADT)
            cc2_out = dram.tile([N * LPAD], ADT)
            nc.scalar.dma_start(
                cc2_in[:].rearrange("(p f) -> p f", p=P),
                l1c[:].rearrange("p n f -> p (n f)"))
            nc.gpsimd.collective_compute(
                "AllGather", ALU.bypass, replica_groups=rg,
                ins=[cc2_in.opt()], outs=[cc2_out.opt()])
            nc.scalar.dma_start(
                l1_rhs[:].rearrange("p c f -> p (c f)")
                .rearrange("p (k f) -> p k f", k=NCORES),
                cc2_out[:].rearrange("(k p f) -> p k f", k=NCORES, p=P))

            # ---- propagation pass 2: y2 = adj @ l1 ------------------------
            with tc.tile_pool(name="y2ps", bufs=1, space="PSUM") as y2psp:
                y2ps = [y2psp.tile([NCLASS, ISL], F32, name=f"y2ps{i}",
                                   tag=f"y2ps{i}") for i in range(NISL)]
                for b in range(nblk):
                    tt = ttp.tile([P, NB, ROWS], ADT, name="tt", tag="tt")
                    nc.sync.dma_start(tt[:], t_blk[b])
                    for s2 in range(NB // 2):
                        jc = b * NB + 2 * s2
                        for i in range(NISL):
                            nc.tensor.matmul(
                                y2ps[i][:], l1_rhs[:, jc:jc + 2, 0:NCLASS],
                                tt[:, 2 * s2:2 * s2 + 2,
                                   i * ISL:(i + 1) * ISL],
                                start=(jc == 0), stop=(jc == CHUNKS - 2),
                                perf_mode=mybir.MatmulPerfMode.DoubleRow)

                y2T = persist.tile([NCLASS, ROWS], F32)
                for i in range(NISL):
                    nc.scalar.activation(y2T[:, i * ISL:(i + 1) * ISL],
                                         y2ps[i][:], AF.Copy)

            # ---- final update + log_softmax -------------------------------
            with (
                tc.tile_pool(name="fin", bufs=1) as fin,
                tc.tile_pool(name="finps", bufs=1, space="PSUM") as finps,
            ):
                y2tp = finps.tile([P, LCH, NCLASS], F32)
                for n in range(LCH):
                    nc.tensor.transpose(y2tp[:, n, :],
                                        y2T[:, n * P:(n + 1) * P],
                                        eye_sb[0:NCLASS, 0:NCLASS])
                lg = fin.tile([P, LCH, NCLASS], F32)
                nc.vector.tensor_mul(lg[:], y2tp[:],
                                     d_all[:].broadcast_to([P, LCH, NCLASS]))
                nc.vector.tensor_add(lg[:], lg[:], alpha_l0[:])
                negm = fin.tile([P, LCH], F32)
                nc.vector.tensor_reduce(negm[:], lg[:], axis=AX.X, op=ALU.max,
                                        negate=True)
                lgm = fin.tile([P, LCH, NCLASS], F32)
                nc.vector.tensor_add(lgm[:], lg[:],
                                     negm[:].broadcast_to([P, LCH, NCLASS]))
                ex = fin.tile([P, LCH, NCLASS], F32)
                nc.scalar.activation(ex[:], lgm[:], AF.Exp)
                sm = fin.tile([P, LCH], F32)
                nc.vector.tensor_reduce(sm[:], ex[:], axis=AX.X, op=ALU.add)
                rs = fin.tile([P, LCH], F32)
                nc.vector.reciprocal(rs[:], sm[:])
                nls = fin.tile([P, LCH], F32)
                nc.scalar.activation(nls[:], rs[:], AF.Ln)
                nc.vector.tensor_add(out_sb[:], lgm[:],
                                     nls[:].broadcast_to([P, LCH, NCLASS]))

            nc.scalar.dma_start(out_d[:],
                                out_sb[:].rearrange("p n f -> p (n f)"))

    nc.compile()
    return nc


def kernel(input, adj, W1, W2):
    """Full inputs in, full [N, NCLASS] float32 log-softmax out."""
    global _COMPILED, LAST_EXEC_TIME_NS, LAST_RESULTS
    if _COMPILED is None:
        _COMPILED = _build()
    nc = _COMPILED

    input = np.asarray(input, dtype=np.float32)
    adj = np.asarray(adj, dtype=np.float32)
    W1 = np.asarray(W1, dtype=np.float32)
    W2 = np.asarray(W2, dtype=np.float32)

    adj_q = adj.astype(ADT_NP)
    xt = np.ascontiguousarray(input.T)
    eye = np.eye(4, dtype=np.float32)

    in_maps = []
    for r in range(NCORES):
        t_r = np.ascontiguousarray(adj_q[r * ROWS:(r + 1) * ROWS, :].T)
        in_maps.append({
            "t": t_r,
            "xt": np.ascontiguousarray(xt[:, r * ROWS:(r + 1) * ROWS]),
            "w1": W1,
            "w2": W2,
            "eye": eye,
        })

    res = bass_utils.run_bass_kernel_spmd(
        nc, in_maps, core_ids=list(range(NCORES)),
        trace=bool(os.environ.get("GNN_TRACE")))
    LAST_EXEC_TIME_NS = res.exec_time_ns
    LAST_RESULTS = res

    out = np.empty((N, NCLASS), dtype=np.float32)
    for r in range(NCORES):
        blk = res.results[r]["out"].reshape(P, LCH, NCLASS)
        out[r * ROWS:(r + 1) * ROWS] = (
            blk.transpose(1, 0, 2).reshape(ROWS, NCLASS))
    return out


# revision 11
# speedup vs baseline: 2.0239x; 1.0928x over previous
"""GCN + 2-step APPNP propagation on 8 Trainium2 NeuronCores.

Reference computation (N=16384, NFEAT=500, HIDDEN=32, NCLASS=3, alpha=0.25):
    h   = relu(input @ W1)
    l0  = h @ W2
    deg = adj.sum(axis=1);  d = (1 - alpha) / max(deg, 1e-12)
    l1  = d * (adj @ l0) + alpha * l0
    l2  = d * (adj @ l1) + alpha * l0
    out = log_softmax(l2, axis=1)

Distribution: 1D row partition of the graph; core r owns rows
r*2048..(r+1)*2048.  The dominant cost is streaming adj twice.

Layout: TensorE contracts over the partition axis, so adj @ L needs adj's
column index on partitions; each core gets T_r = adj[rows_r, :].T
([N, 2048] row-major), quantized to fp8-e4m3 on the host (4x less HBM
traffic than fp32; measured output error ~1e-4 relative because the
propagated term is small next to the fp32 alpha*l0 term and quantization
noise averages over 16k-term dot products).  A [128, c] chunk of L is the
stationary operand (LDWEIGHTS ~free); T_r streams as the moving operand.

deg rides along pass 1 as a ones-column of L0.  Between passes the tiny
per-core logits are AllGathered through a DRAM bounce.  Small/latency-
critical DMAs go on the scalar-engine HWDGE queue so they never sit
behind the 2 MiB stream DMAs on the sync queue.  Output leaves in a
chunk-major [128, 16, 3] layout and is un-permuted on the host.
"""

import os

import numpy as np
import ml_dtypes

import concourse.bass as bass
import concourse.mybir as mybir
import concourse.bacc as bacc
import concourse.tile as tile
from concourse import bass_utils

N = 16384
NFEAT = 500
HIDDEN = 32
NCLASS = 3
ALPHA = 0.25
NCORES = 8
ROWS = N // NCORES        # 2048 rows owned per core
P = 128                   # SBUF partitions
CHUNKS = N // P           # 128 global row-chunks
LCH = ROWS // P           # 16 local row-chunks
NB = 8                    # row-chunks per adj DMA block
ISL = 512                 # moving-operand free-dim per matmul
NISL = ROWS // ISL        # 4 output column slices
TT_BUFS = 9               # adj stream prefetch depth (x2 MiB)
LPAD = 16                 # L-chunk stride (DoubleRow needs step%16==0)

F32 = mybir.dt.float32
BF16 = mybir.dt.bfloat16
ADT = mybir.dt.float8e4
ADT_NP = ml_dtypes.float8_e4m3
BF16_NP = ml_dtypes.bfloat16
AF = mybir.ActivationFunctionType
ALU = mybir.AluOpType
AX = mybir.AxisListType

_COMPILED = None
LAST_EXEC_TIME_NS = None
LAST_RESULTS = None


def _build():
    nc = bacc.Bacc("TRN2", target_bir_lowering=False, debug=False,
                   num_devices=NCORES)

    t_d = nc.dram_tensor("t", [N, ROWS], ADT, kind="ExternalInput").ap()
    xt_d = nc.dram_tensor("xt", [NFEAT, ROWS], BF16, kind="ExternalInput").ap()
    w1_d = nc.dram_tensor("w1", [NFEAT, HIDDEN], BF16, kind="ExternalInput").ap()
    w2_d = nc.dram_tensor("w2", [HIDDEN, NCLASS], F32, kind="ExternalInput").ap()
    eye_d = nc.dram_tensor("eye", [4, 4], F32, kind="ExternalInput").ap()
    out_d = nc.dram_tensor("out", [P, LCH * NCLASS], F32,
                           kind="ExternalOutput").ap()

    # adj^T shard viewed as DMA blocks: row = b*(NB*P) + s*P + p
    t_blk = t_d.rearrange("(b s p) f -> b p s f", p=P, s=NB)
    nblk = N // (NB * P)  # 16

    rg = [list(range(NCORES))]

    with tile.TileContext(nc) as tc:
        with (
            tc.tile_pool(name="const", bufs=1) as const,
            tc.tile_pool(name="persist", bufs=1) as persist,
            tc.tile_pool(name="ttp", bufs=TT_BUFS) as ttp,
            tc.tile_pool(name="dram", bufs=1, space="DRAM") as dram,
        ):
            eye_sb = const.tile([4, 4], F32)
            nc.scalar.dma_start(eye_sb[:], eye_d[:])
            w2_sb = const.tile([HIDDEN, NCLASS], F32)
            nc.scalar.dma_start(w2_sb[:], w2_d[:])

            # live across the whole kernel
            alpha_l0 = persist.tile([P, LCH, NCLASS], F32)    # 0.25*l0, local
            d_all = persist.tile([P, LCH], F32)               # 0.75/deg, local
            l0_rhs = persist.tile([P, CHUNKS, LPAD], ADT)     # [l0 | 1] chunks
            l1_rhs = persist.tile([P, CHUNKS, LPAD], ADT)     # l1 chunks
            l0c = persist.tile([P, LCH, LPAD], ADT)           # local AG payload
            l1c = persist.tile([P, LCH, LPAD], ADT)           # local AG payload
            out_sb = persist.tile([P, LCH, NCLASS], F32)

            # ---- stage 1: local l0 = relu(x @ W1) @ W2 (transposed forms) --
            ksz = [P, P, P, NFEAT - 3 * P]  # 500 = 128*3 + 116
            with (
                tc.tile_pool(name="s1sb", bufs=1) as s1sb,
                tc.tile_pool(name="s1ps", bufs=2, space="PSUM") as s1ps,
                tc.tile_pool(name="l0psp", bufs=1, space="PSUM") as l0psp,
            ):
                w1c, xtc = [], []
                for k in range(4):
                    w = s1sb.tile([ksz[k], HIDDEN], BF16, name=f"w1c{k}")
                    nc.sync.dma_start(w[:], w1_d[k * P:k * P + ksz[k], :])
                    w1c.append(w)
                for k in range(4):
                    x = s1sb.tile([ksz[k], ROWS], BF16, name=f"xtc{k}")
                    nc.sync.dma_start(x[:], xt_d[k * P:k * P + ksz[k], :])
                    xtc.append(x)

                hT = s1sb.tile([HIDDEN, ROWS], F32)  # h^T in SBUF
                for i in range(NISL):
                    hps = s1ps.tile([HIDDEN, ISL], F32, name=f"hps{i}",
                                    tag="hps")
                    for k in range(4):
                        nc.tensor.matmul(
                            hps[:], w1c[k][:],
                            xtc[k][:, i * ISL:(i + 1) * ISL],
                            start=(k == 0), stop=(k == 3))
                    nc.scalar.activation(hT[:, i * ISL:(i + 1) * ISL], hps[:],
                                         AF.Relu)

                l0ps = l0psp.tile([P, LCH, NCLASS], F32)
                for n in range(LCH):
                    nc.tensor.matmul(l0ps[:, n, :], hT[:, n * P:(n + 1) * P],
                                     w2_sb[:], start=True, stop=True)
                nc.vector.tensor_scalar_mul(alpha_l0[:], l0ps[:], ALPHA)
                nc.scalar.activation(l0c[:, :, 0:NCLASS], l0ps[:], AF.Copy)
                nc.vector.memset(l0c[:, :, NCLASS], 1.0)
                nc.vector.memset(l0c[:, :, NCLASS + 1:LPAD], 0.0)

            # ---- all-gather l0 (with ones col) into every core's rhs ------
            cc1_in = dram.tile([ROWS * LPAD], ADT)
            cc1_out = dram.tile([N * LPAD], ADT)
            nc.scalar.dma_start(
                cc1_in[:].rearrange("(p f) -> p f", p=P),
                l0c[:].rearrange("p n f -> p (n f)"))
            nc.gpsimd.collective_compute(
                "AllGather", ALU.bypass, replica_groups=rg,
                ins=[cc1_in.opt()], outs=[cc1_out.opt()])
            nc.scalar.dma_start(
                l0_rhs[:].rearrange("p c f -> p (c f)")
                .rearrange("p (k f) -> p k f", k=NCORES),
                cc1_out[:].rearrange("(k p f) -> p k f", k=NCORES, p=P))

            # ---- propagation pass 1: y1 = adj @ [l0 | 1] ------------------
            with tc.tile_pool(name="y1ps", bufs=1, space="PSUM") as y1psp:
                y1ps = [y1psp.tile([4, ISL], F32, name=f"y1ps{i}",
                                   tag=f"y1ps{i}") for i in range(NISL)]
                for b in range(nblk):
                    tt = ttp.tile([P, NB, ROWS], ADT, name="tt", tag="tt")
                    nc.sync.dma_start(tt[:], t_blk[b])
                    for s2 in range(NB // 2):
                        jc = b * NB + 2 * s2
                        for i in range(NISL):
                            nc.tensor.matmul(
                                y1ps[i][:], l0_rhs[:, jc:jc + 2, 0:4],
                                tt[:, 2 * s2:2 * s2 + 2,
                                   i * ISL:(i + 1) * ISL],
                                start=(jc == 0), stop=(jc == CHUNKS - 2),
                                perf_mode=mybir.MatmulPerfMode.DoubleRow)

                y1T = persist.tile([4, ROWS], F32)
                for i in range(NISL):
                    nc.scalar.activation(y1T[:, i * ISL:(i + 1) * ISL],
                                         y1ps[i][:], AF.Copy)

            # ---- iteration update: l1 = d*y1 + alpha*l0 -------------------
            with (
                tc.tile_pool(name="upd", bufs=1) as upd,
                tc.tile_pool(name="updps", bufs=1, space="PSUM") as updps,
            ):
                ytp = updps.tile([P, LCH, 4], F32)
                for n in range(LCH):
                    nc.tensor.transpose(ytp[:, n, :],
                                        y1T[:, n * P:(n + 1) * P], eye_sb[:])
                dmx = upd.tile([P, LCH], F32)
                nc.vector.tensor_scalar_max(dmx[:], ytp[:, :, 3], 1e-12)
                rec = upd.tile([P, LCH], F32)
                nc.vector.reciprocal(rec[:], dmx[:])
                nc.vector.tensor_scalar_mul(d_all[:], rec[:], 1.0 - ALPHA)
                ty = upd.tile([P, LCH, NCLASS], F32)
                nc.vector.tensor_mul(ty[:], ytp[:, :, 0:NCLASS],
                                     d_all[:].broadcast_to([P, LCH, NCLASS]))
                nc.vector.tensor_add(l1c[:, :, 0:NCLASS], ty[:],
                                     alpha_l0[:])
                nc.vector.memset(l1c[:, :, NCLASS:LPAD], 0.0)

            # ---- all-gather l1 --------------------------------------------
            cc2_in = dram.tile([ROWS * LPAD], ADT)
            cc2_out = dram.tile([N * LPAD], ADT)
            nc.scalar.dma_start(
                cc2_in[:].rearrange("(p f) -> p f", p=P),
                l1c[:].rearrange("p n f -> p (n f)"))
            nc.gpsimd.collective_compute(
                "AllGather", ALU.bypass, replica_groups=rg,
                ins=[cc2_in.opt()], outs=[cc2_out.opt()])
            nc.scalar.dma_start(
                l1_rhs[:].rearrange("p c f -> p (c f)")
                .rearrange("p (k f) -> p k f", k=NCORES),
                cc2_out[:].rearrange("(k p f) -> p k f", k=NCORES, p=P))

            # ---- propagation pass 2: y2 = adj @ l1 ------------------------
            with tc.tile_pool(name="y2ps", bufs=1, space="PSUM") as y2psp:
                y2ps = [y2psp.tile([NCLASS, ISL], F32, name=f"y2ps{i}",
                                   tag=f"y2ps{i}") for i in range(NISL)]
                for b in range(nblk):
                    tt = ttp.tile([P, NB, ROWS], ADT, name="tt", tag="tt")
                    nc.sync.dma_start(tt[:], t_blk[b])
                    for s2 in range(NB // 2):
                        jc = b * NB + 2 * s2
                        for i in range(NISL):
                            nc.tensor.matmul(
                                y2ps[i][:], l1_rhs[:, jc:jc + 2, 0:NCLASS],
                                tt[:, 2 * s2:2 * s2 + 2,
                                   i * ISL:(i + 1) * ISL],
                                start=(jc == 0), stop=(jc == CHUNKS - 2),
                                perf_mode=mybir.MatmulPerfMode.DoubleRow)

                y2T = persist.tile([NCLASS, ROWS], F32)
                for i in range(NISL):
                    nc.scalar.activation(y2T[:, i * ISL:(i + 1) * ISL],
                                         y2ps[i][:], AF.Copy)

            # ---- final update + log_softmax -------------------------------
            with (
                tc.tile_pool(name="fin", bufs=1) as fin,
                tc.tile_pool(name="finps", bufs=1, space="PSUM") as finps,
            ):
                y2tp = finps.tile([P, LCH, NCLASS], F32)
                for n in range(LCH):
                    nc.tensor.transpose(y2tp[:, n, :],
                                        y2T[:, n * P:(n + 1) * P],
                                        eye_sb[0:NCLASS, 0:NCLASS])
                lg = fin.tile([P, LCH, NCLASS], F32)
                nc.vector.tensor_mul(lg[:], y2tp[:],
                                     d_all[:].broadcast_to([P, LCH, NCLASS]))
                nc.vector.tensor_add(lg[:], lg[:], alpha_l0[:])
                negm = fin.tile([P, LCH], F32)
                nc.vector.tensor_reduce(negm[:], lg[:], axis=AX.X, op=ALU.max,
                                        negate=True)
                lgm = fin.tile([P, LCH, NCLASS], F32)
                nc.vector.tensor_add(lgm[:], lg[:],
                                     negm[:].broadcast_to([P, LCH, NCLASS]))
                ex = fin.tile([P, LCH, NCLASS], F32)
                nc.scalar.activation(ex[:], lgm[:], AF.Exp)
                sm = fin.tile([P, LCH], F32)
                nc.vector.tensor_reduce(sm[:], ex[:], axis=AX.X, op=ALU.add)
                rs = fin.tile([P, LCH], F32)
                nc.vector.reciprocal(rs[:], sm[:])
                nls = fin.tile([P, LCH], F32)
                nc.scalar.activation(nls[:], rs[:], AF.Ln)
                nc.vector.tensor_add(out_sb[:], lgm[:],
                                     nls[:].broadcast_to([P, LCH, NCLASS]))

            nc.scalar.dma_start(out_d[:],
                                out_sb[:].rearrange("p n f -> p (n f)"))

    nc.compile()
    return nc


def kernel(input, adj, W1, W2):
    """Full inputs in, full [N, NCLASS] float32 log-softmax out."""
    global _COMPILED, LAST_EXEC_TIME_NS, LAST_RESULTS
    if _COMPILED is None:
        _COMPILED = _build()
    nc = _COMPILED

    input = np.asarray(input, dtype=np.float32)
    adj = np.asarray(adj, dtype=np.float32)
    W1 = np.asarray(W1, dtype=np.float32)
    W2 = np.asarray(W2, dtype=np.float32)

    adj_q = adj.astype(ADT_NP)
    xt = np.ascontiguousarray(input.T).astype(BF16_NP)
    w1_q = W1.astype(BF16_NP)
    eye = np.eye(4, dtype=np.float32)

    in_maps = []
    for r in range(NCORES):
        t_r = np.ascontiguousarray(adj_q[r * ROWS:(r + 1) * ROWS, :].T)
        in_maps.append({
            "t": t_r,
            "xt": np.ascontiguousarray(xt[:, r * ROWS:(r + 1) * ROWS]),
            "w1": w1_q,
            "w2": W2,
            "eye": eye,
        })

    res = bass_utils.run_bass_kernel_spmd(
        nc, in_maps, core_ids=list(range(NCORES)),
        trace=bool(os.environ.get("GNN_TRACE")))
    LAST_EXEC_TIME_NS = res.exec_time_ns
    LAST_RESULTS = res

    out = np.empty((N, NCLASS), dtype=np.float32)
    for r in range(NCORES):
        blk = res.results[r]["out"].reshape(P, LCH, NCLASS)
        out[r * ROWS:(r + 1) * ROWS] = (
            blk.transpose(1, 0, 2).reshape(ROWS, NCLASS))
    return out


# revision 12
# speedup vs baseline: 2.1282x; 1.0515x over previous
"""GCN + 2-step APPNP propagation on 8 Trainium2 NeuronCores.

Reference computation (N=16384, NFEAT=500, HIDDEN=32, NCLASS=3, alpha=0.25):
    h   = relu(input @ W1)
    l0  = h @ W2
    deg = adj.sum(axis=1);  d = (1 - alpha) / max(deg, 1e-12)
    l1  = d * (adj @ l0) + alpha * l0
    l2  = d * (adj @ l1) + alpha * l0
    out = log_softmax(l2, axis=1)

Distribution: 1D row partition of the graph; core r owns rows
r*2048..(r+1)*2048.  The dominant cost is streaming adj twice.

Layout: TensorE contracts over the partition axis, so adj @ L needs adj's
column index on partitions; each core gets T_r = adj[rows_r, :].T
([N, 2048] row-major), quantized to fp8-e4m3 on the host (4x less HBM
traffic than fp32; measured output error ~1e-4 relative because the
propagated term is small next to the fp32 alpha*l0 term and quantization
noise averages over 16k-term dot products).  A [128, c] chunk of L is the
stationary operand (LDWEIGHTS ~free); T_r streams as the moving operand.

deg rides along pass 1 as a ones-column of L0.  Between passes the tiny
per-core logits are AllGathered through a DRAM bounce.  Small/latency-
critical DMAs go on the scalar-engine HWDGE queue so they never sit
behind the 2 MiB stream DMAs on the sync queue.  Output leaves in a
chunk-major [128, 16, 3] layout and is un-permuted on the host.
"""

import os

import numpy as np
import ml_dtypes

import concourse.bass as bass
import concourse.mybir as mybir
import concourse.bacc as bacc
import concourse.tile as tile
from concourse import bass_utils

N = 16384
NFEAT = 500
HIDDEN = 32
NCLASS = 3
ALPHA = 0.25
NCORES = 8
ROWS = N // NCORES        # 2048 rows owned per core
P = 128                   # SBUF partitions
CHUNKS = N // P           # 128 global row-chunks
LCH = ROWS // P           # 16 local row-chunks
NB = 8                    # row-chunks per adj DMA block
ISL = 512                 # moving-operand free-dim per matmul
NISL = ROWS // ISL        # 4 output column slices
TT_BUFS = 9               # adj stream prefetch depth (x2 MiB)
LPAD = 16                 # L-chunk stride (DoubleRow needs step%16==0)

F32 = mybir.dt.float32
BF16 = mybir.dt.bfloat16
ADT = mybir.dt.float8e4
ADT_NP = ml_dtypes.float8_e4m3
BF16_NP = ml_dtypes.bfloat16
AF = mybir.ActivationFunctionType
ALU = mybir.AluOpType
AX = mybir.AxisListType

_COMPILED = None
LAST_EXEC_TIME_NS = None
LAST_RESULTS = None


def _build():
    nc = bacc.Bacc("TRN2", target_bir_lowering=False, debug=False,
                   num_devices=NCORES)

    t_d = nc.dram_tensor("t", [N // (NB * P), P, NB * ROWS], ADT,
                         kind="ExternalInput").ap()
    xt_d = nc.dram_tensor("xt", [NFEAT, ROWS], BF16, kind="ExternalInput").ap()
    w1_d = nc.dram_tensor("w1", [NFEAT, HIDDEN], BF16, kind="ExternalInput").ap()
    w2_d = nc.dram_tensor("w2", [HIDDEN, NCLASS], F32, kind="ExternalInput").ap()
    eye_d = nc.dram_tensor("eye", [4, 4], F32, kind="ExternalInput").ap()
    out_d = nc.dram_tensor("out", [P, LCH * NCLASS], F32,
                           kind="ExternalOutput").ap()

    nblk = N // (NB * P)  # 16

    rg = [list(range(NCORES))]

    with tile.TileContext(nc) as tc:
        with (
            tc.tile_pool(name="const", bufs=1) as const,
            tc.tile_pool(name="persist", bufs=1) as persist,
            tc.tile_pool(name="ttp", bufs=TT_BUFS) as ttp,
            tc.tile_pool(name="dram", bufs=1, space="DRAM") as dram,
        ):
            eye_sb = const.tile([4, 4], F32)
            nc.gpsimd.dma_start(eye_sb[:], eye_d[:])
            w2_sb = const.tile([HIDDEN, NCLASS], F32)
            nc.gpsimd.dma_start(w2_sb[:], w2_d[:])

            # live across the whole kernel
            alpha_l0 = persist.tile([P, LCH, NCLASS], F32)    # 0.25*l0, local
            d_all = persist.tile([P, LCH], F32)               # 0.75/deg, local
            l0_rhs = persist.tile([P, CHUNKS, LPAD], ADT)     # [l0 | 1] chunks
            l1_rhs = persist.tile([P, CHUNKS, LPAD], ADT)     # l1 chunks
            l0c = persist.tile([P, LCH, LPAD], ADT)           # local AG payload
            l1c = persist.tile([P, LCH, LPAD], ADT)           # local AG payload
            out_sb = persist.tile([P, LCH, NCLASS], F32)

            # ---- stage 1: local l0 = relu(x @ W1) @ W2 (transposed forms) --
            ksz = [P, P, P, NFEAT - 3 * P]  # 500 = 128*3 + 116
            with (
                tc.tile_pool(name="s1sb", bufs=1) as s1sb,
                tc.tile_pool(name="s1ps", bufs=2, space="PSUM") as s1ps,
                tc.tile_pool(name="l0psp", bufs=1, space="PSUM") as l0psp,
            ):
                w1c, xtc = [], []
                for k in range(4):
                    w = s1sb.tile([ksz[k], HIDDEN], BF16, name=f"w1c{k}")
                    nc.sync.dma_start(w[:], w1_d[k * P:k * P + ksz[k], :])
                    w1c.append(w)
                for k in range(4):
                    x = s1sb.tile([ksz[k], ROWS], BF16, name=f"xtc{k}")
                    nc.sync.dma_start(x[:], xt_d[k * P:k * P + ksz[k], :])
                    xtc.append(x)

                hT = s1sb.tile([HIDDEN, ROWS], F32)  # h^T in SBUF
                for i in range(NISL):
                    hps = s1ps.tile([HIDDEN, ISL], F32, name=f"hps{i}",
                                    tag="hps")
                    for k in range(4):
                        nc.tensor.matmul(
                            hps[:], w1c[k][:],
                            xtc[k][:, i * ISL:(i + 1) * ISL],
                            start=(k == 0), stop=(k == 3))
                    nc.scalar.activation(hT[:, i * ISL:(i + 1) * ISL], hps[:],
                                         AF.Relu)

                l0ps = l0psp.tile([P, LCH, NCLASS], F32)
                for n in range(LCH):
                    nc.tensor.matmul(l0ps[:, n, :], hT[:, n * P:(n + 1) * P],
                                     w2_sb[:], start=True, stop=True)
                nc.vector.tensor_scalar_mul(alpha_l0[:], l0ps[:], ALPHA)
                nc.scalar.activation(l0c[:, :, 0:NCLASS], l0ps[:], AF.Copy)
                nc.vector.memset(l0c[:, :, NCLASS], 1.0)
                nc.vector.memset(l0c[:, :, NCLASS + 1:LPAD], 0.0)

            # ---- all-gather l0 (with ones col) into every core's rhs ------
            cc1_in = dram.tile([ROWS * LPAD], ADT)
            cc1_out = dram.tile([N * LPAD], ADT)
            nc.gpsimd.dma_start(
                cc1_in[:].rearrange("(p f) -> p f", p=P),
                l0c[:].rearrange("p n f -> p (n f)"))
            nc.gpsimd.collective_compute(
                "AllGather", ALU.bypass, replica_groups=rg,
                ins=[cc1_in.opt()], outs=[cc1_out.opt()])
            nc.gpsimd.dma_start(
                l0_rhs[:].rearrange("p c f -> p (c f)")
                .rearrange("p (k f) -> p k f", k=NCORES),
                cc1_out[:].rearrange("(k p f) -> p k f", k=NCORES, p=P))

            # ---- propagation pass 1: y1 = adj @ [l0 | 1] ------------------
            with tc.tile_pool(name="y1ps", bufs=1, space="PSUM") as y1psp:
                y1ps = [y1psp.tile([4, ISL], F32, name=f"y1ps{i}",
                                   tag=f"y1ps{i}") for i in range(NISL)]
                for b in range(nblk):
                    tt = ttp.tile([P, NB * ROWS], ADT, name="tt", tag="tt")
                    nc.sync.dma_start(tt[:], t_d[b])
                    tt3 = tt[:].rearrange("p (s f) -> p s f", s=NB)
                    for s2 in range(NB // 2):
                        jc = b * NB + 2 * s2
                        for i in range(NISL):
                            nc.tensor.matmul(
                                y1ps[i][:], l0_rhs[:, jc:jc + 2, 0:4],
                                tt3[:, 2 * s2:2 * s2 + 2,
                                    i * ISL:(i + 1) * ISL],
                                start=(jc == 0), stop=(jc == CHUNKS - 2),
                                perf_mode=mybir.MatmulPerfMode.DoubleRow)

                y1T = persist.tile([4, ROWS], F32)
                for i in range(NISL):
                    nc.scalar.activation(y1T[:, i * ISL:(i + 1) * ISL],
                                         y1ps[i][:], AF.Copy)

            # ---- iteration update: l1 = d*y1 + alpha*l0 -------------------
            with (
                tc.tile_pool(name="upd", bufs=1) as upd,
                tc.tile_pool(name="updps", bufs=1, space="PSUM") as updps,
            ):
                ytp = updps.tile([P, LCH, 4], F32)
                for n in range(LCH):
                    nc.tensor.transpose(ytp[:, n, :],
                                        y1T[:, n * P:(n + 1) * P], eye_sb[:])
                dmx = upd.tile([P, LCH], F32)
                nc.vector.tensor_scalar_max(dmx[:], ytp[:, :, 3], 1e-12)
                rec = upd.tile([P, LCH], F32)
                nc.vector.reciprocal(rec[:], dmx[:])
                nc.vector.tensor_scalar_mul(d_all[:], rec[:], 1.0 - ALPHA)
                ty = upd.tile([P, LCH, NCLASS], F32)
                nc.vector.tensor_mul(ty[:], ytp[:, :, 0:NCLASS],
                                     d_all[:].broadcast_to([P, LCH, NCLASS]))
                nc.vector.tensor_add(l1c[:, :, 0:NCLASS], ty[:],
                                     alpha_l0[:])
                nc.vector.memset(l1c[:, :, NCLASS:LPAD], 0.0)

            # ---- all-gather l1 --------------------------------------------
            cc2_in = dram.tile([ROWS * LPAD], ADT)
            cc2_out = dram.tile([N * LPAD], ADT)
            nc.gpsimd.dma_start(
                cc2_in[:].rearrange("(p f) -> p f", p=P),
                l1c[:].rearrange("p n f -> p (n f)"))
            nc.gpsimd.collective_compute(
                "AllGather", ALU.bypass, replica_groups=rg,
                ins=[cc2_in.opt()], outs=[cc2_out.opt()])
            nc.gpsimd.dma_start(
                l1_rhs[:].rearrange("p c f -> p (c f)")
                .rearrange("p (k f) -> p k f", k=NCORES),
                cc2_out[:].rearrange("(k p f) -> p k f", k=NCORES, p=P))

            # ---- propagation pass 2: y2 = adj @ l1 ------------------------
            with tc.tile_pool(name="y2ps", bufs=1, space="PSUM") as y2psp:
                y2ps = [y2psp.tile([NCLASS, ISL], F32, name=f"y2ps{i}",
                                   tag=f"y2ps{i}") for i in range(NISL)]
                for b in range(nblk):
                    tt = ttp.tile([P, NB * ROWS], ADT, name="tt", tag="tt")
                    nc.sync.dma_start(tt[:], t_d[b])
                    tt3 = tt[:].rearrange("p (s f) -> p s f", s=NB)
                    for s2 in range(NB // 2):
                        jc = b * NB + 2 * s2
                        for i in range(NISL):
                            nc.tensor.matmul(
                                y2ps[i][:], l1_rhs[:, jc:jc + 2, 0:NCLASS],
                                tt3[:, 2 * s2:2 * s2 + 2,
                                    i * ISL:(i + 1) * ISL],
                                start=(jc == 0), stop=(jc == CHUNKS - 2),
                                perf_mode=mybir.MatmulPerfMode.DoubleRow)

                y2T = persist.tile([NCLASS, ROWS], F32)
                for i in range(NISL):
                    nc.scalar.activation(y2T[:, i * ISL:(i + 1) * ISL],
                                         y2ps[i][:], AF.Copy)

            # ---- final update + log_softmax -------------------------------
            with (
                tc.tile_pool(name="fin", bufs=1) as fin,
                tc.tile_pool(name="finps", bufs=1, space="PSUM") as finps,
            ):
                y2tp = finps.tile([P, LCH, NCLASS], F32)
                for n in range(LCH):
                    nc.tensor.transpose(y2tp[:, n, :],
                                        y2T[:, n * P:(n + 1) * P],
                                        eye_sb[0:NCLASS, 0:NCLASS])
                lg = fin.tile([P, LCH, NCLASS], F32)
                nc.vector.tensor_mul(lg[:], y2tp[:],
                                     d_all[:].broadcast_to([P, LCH, NCLASS]))
                nc.vector.tensor_add(lg[:], lg[:], alpha_l0[:])
                negm = fin.tile([P, LCH], F32)
                nc.vector.tensor_reduce(negm[:], lg[:], axis=AX.X, op=ALU.max,
                                        negate=True)
                lgm = fin.tile([P, LCH, NCLASS], F32)
                nc.vector.tensor_add(lgm[:], lg[:],
                                     negm[:].broadcast_to([P, LCH, NCLASS]))
                ex = fin.tile([P, LCH, NCLASS], F32)
                nc.scalar.activation(ex[:], lgm[:], AF.Exp)
                sm = fin.tile([P, LCH], F32)
                nc.vector.tensor_reduce(sm[:], ex[:], axis=AX.X, op=ALU.add)
                rs = fin.tile([P, LCH], F32)
                nc.vector.reciprocal(rs[:], sm[:])
                nls = fin.tile([P, LCH], F32)
                nc.scalar.activation(nls[:], rs[:], AF.Ln)
                nc.vector.tensor_add(out_sb[:], lgm[:],
                                     nls[:].broadcast_to([P, LCH, NCLASS]))

            nc.gpsimd.dma_start(out_d[:],
                                out_sb[:].rearrange("p n f -> p (n f)"))

    nc.compile()
    return nc


def kernel(input, adj, W1, W2):
    """Full inputs in, full [N, NCLASS] float32 log-softmax out."""
    global _COMPILED, LAST_EXEC_TIME_NS, LAST_RESULTS
    if _COMPILED is None:
        _COMPILED = _build()
    nc = _COMPILED

    input = np.asarray(input, dtype=np.float32)
    adj = np.asarray(adj, dtype=np.float32)
    W1 = np.asarray(W1, dtype=np.float32)
    W2 = np.asarray(W2, dtype=np.float32)

    adj_q = adj.astype(ADT_NP)
    xt = np.ascontiguousarray(input.T).astype(BF16_NP)
    w1_q = W1.astype(BF16_NP)
    eye = np.eye(4, dtype=np.float32)

    in_maps = []
    for r in range(NCORES):
        t_r = np.ascontiguousarray(
            adj_q[r * ROWS:(r + 1) * ROWS, :].T
            .reshape(N // (NB * P), NB, P, ROWS)
            .transpose(0, 2, 1, 3)
            .reshape(N // (NB * P), P, NB * ROWS))
        in_maps.append({
            "t": t_r,
            "xt": np.ascontiguousarray(xt[:, r * ROWS:(r + 1) * ROWS]),
            "w1": w1_q,
            "w2": W2,
            "eye": eye,
        })

    res = bass_utils.run_bass_kernel_spmd(
        nc, in_maps, core_ids=list(range(NCORES)),
        trace=bool(os.environ.get("GNN_TRACE")))
    LAST_EXEC_TIME_NS = res.exec_time_ns
    LAST_RESULTS = res

    out = np.empty((N, NCLASS), dtype=np.float32)
    for r in range(NCORES):
        blk = res.results[r]["out"].reshape(P, LCH, NCLASS)
        out[r * ROWS:(r + 1) * ROWS] = (
            blk.transpose(1, 0, 2).reshape(ROWS, NCLASS))
    return out
